# revision 29
# baseline (speedup 1.0000x reference)
"""Multi-head causal attention (B=2, T=2048, E=1024, H=16, D=64) on 8 TRN2
NeuronCores, tensor-parallel over heads (2 heads per core).

All matmul operands are bf16 (same PE rate as fp32r for N>=256 columns, but
no fp32r 4x penalty on the <256-col causally-trimmed matmuls; halves
DMA/SBUF; x and y travel as bf16). Host-simulated precision of the all-bf16
pipeline: ~5e-3 rel err vs the 2e-2 gate. Measured: 200.3us / 4.03e-3
(baseline fp32r version: 257.7us / 2.8e-4).

Dataflow per core (tensor-parallel over heads; host sums the 8 partial y):
  phase 1 per 512-t block: qT/kT/vT = wqkv^T @ xT (PSUM accum over 8
    e-tiles; q/k in 'acc' banks, vT in a 'wei' bank), then one block later
    PE-transpose vT -> v [s, (sc, h, d)] tiles with a ones column (the
    softmax denominator rides row 64 of the AV matmul output). Weight DMAs
    stream inside block 0; wproj loads after phase 1 emission.
  phase 2 per (b, 512-t block), batches interleaved (b0t0, b1t0, b0t1, ...):
    per 128-s tile x 2 heads: weiT[s,t] = kT^T q (diagonal tiles compute
    only cols >= woff), Exp on ACT (scale=E^-0.5) -> bf16 wt, multiplicative
    tril mask (DVE) on the diagonal chunk, avT[65, t] += [v|1]^T @ wt.
  epilogue per block, decomposed into 11 steps run from a global queue at
    the NEXT blocks' (si, h) sub-boundaries (~1 step per 2 boundaries,
    spilling across blocks; the 3 chain steps must run in the immediate
    next block to free av banks):
      pick: l rows -> t-partitions via a [33,2]-selector matmul, then a DVE
        reciprocal over 8 elements (not 512 -- DVE cost is free-size-bound);
      backT: 8 tiny PE transposes put 1/l back on rows 0/32; bcast: per-head
        K=1 ones matmul broadcasts 1/l over 64 d-partitions (reads only the
        written rows -- rows 1-31 of rc_ps are psum garbage, 0*NaN=NaN);
      avT = av * bc on DVE (bf16), 8 proj matmuls y[t,e] = avT^T @ wproj,
        DVE psum->bf16 copies, DMA out. The final (hostless) epilogue
        alternates its y banks with the idle wei banks.
PSUM banks: av(acc)=4 (2 heads x 2 blocks in flight) + wei=3 + epi(y)=1 = 8.

Infra notes: this container's walrus accepts at most ONE semaphore wait
per instruction (_split_multi_waits hoists extras onto EventSemaphores);
custom-DVE ops / gpsimd partition_broadcast do not compile ("ISA wrong
length"); GPSIMD (Pool) cannot access PSUM and rejects TensorTensor ops
entirely (memset/affine_select only); DMA cannot touch PSUM; non-matmul
engine APs need 32-partition-aligned starts (hence rows 0/32 for l);
matmul lhsT and rhs must share their base partition. The ACT engine must
run a single function (Exp) -- mixing in Copy/Reciprocal thrashes the
activation table at ~1.3us per swap. The PE runs 1.2GHz cold / 2.4GHz
after ~3-4us of sustained work; phase 1 sustains 216ns per 512-col matmul,
the attention phase levels at ~320-335ns (engine-concurrency effect not
worth more gap-chasing below ~300ns granularity).
"""
import sys
import types

import numpy as np

B, T, E, H, D = 2, 2048, 1024, 16, 64
N_CORES = 8
HPC = H // N_CORES          # heads per core = 2
BT = B * T                  # 4096
DPC = HPC * D               # 128 head-dims per core
SCALE = 1.0 / float(np.sqrt(E))  # NOTE: reference scales by E**-0.5

# y copies all on DVE (GPSIMD cannot read PSUM); mask muls go to GpSimd
Y_COPY_ON_GPSIMD = (False,) * 8


def _install_ntff_hook():
    if 'antenv.axon_hooks' in sys.modules:
        return
    try:
        sys.path.insert(0, '/root/.axon_site')
        from trn_agent_boot.trn_boot import _ntff_profile_via_ctypes
        hook = _ntff_profile_via_ctypes('/opt/axon/libaxon_pjrt.so')
        mod = types.ModuleType('antenv.axon_hooks')
        mod.get_axon_ntff_profile_hook = lambda: hook
        mod.set_axon_ntff_profile_hook = lambda h: None
        sys.modules['antenv.axon_hooks'] = mod
    except Exception:
        pass


def _split_multi_waits(nc, mybir):
    """This walrus build rejects >1 sync-wait per instruction. Hoist extra
    waits onto EventSemaphore instructions on the same engine just before."""
    for f in nc.m.functions:
        for bb in f.blocks:
            new_insts = []
            changed = False
            for inst in bb.instructions:
                si = inst.sync_info
                if si is not None and len(si.on_wait) > 1:
                    extra = list(si.on_wait[:-1])
                    keep = si.on_wait[-1]
                    for w in extra:
                        ev = mybir.InstEventSemaphore(
                            name=f"I-{nc.next_id()}", ins=[], outs=[])
                        ev.engine = inst.engine
                        ev.sync_info = mybir.SyncInfo(on_wait=[w], on_update=[])
                        new_insts.append(ev)
                    del si.on_wait[:]
                    si.on_wait.append(keep)
                    changed = True
                new_insts.append(inst)
            if changed:
                bb.instructions = new_insts


def _build_nc():
    import concourse.bass as bass
    import concourse.mybir as mybir
    import concourse.tile as tile
    from concourse.masks import make_identity

    f32 = mybir.dt.float32
    bf16 = mybir.dt.bfloat16
    EXP = mybir.ActivationFunctionType.Exp
    DIV = mybir.AluOpType.divide

    nc = bass.Bass('TRN2', num_devices=N_CORES)
    xt = nc.dram_tensor('xt', [E, BT], bf16, kind='ExternalInput')
    wqkv = nc.dram_tensor('wqkv', [E, 3 * DPC], bf16, kind='ExternalInput')
    wproj = nc.dram_tensor('wproj', [DPC, E], bf16, kind='ExternalInput')
    y = nc.dram_tensor('y', [BT, E], bf16, kind='ExternalOutput')

    NTB1 = BT // 512            # 8 t-blocks in phase 1
    NE = E // 128               # 8 e-tiles
    NTB = T // 512              # 4 t-blocks per batch in phase 2

    with tile.TileContext(nc) as tc:
        with tc.tile_pool(name='consts', bufs=1) as consts, \
             tc.tile_pool(name='big', bufs=1) as big, \
             tc.tile_pool(name='work', bufs=1) as work, \
             tc.tile_pool(name='ps', bufs=1, space='PSUM') as ps:

            # ---- constants ----
            ident_f = consts.tile([128, 128], f32)
            make_identity(nc, ident_f)
            ident = consts.tile([128, 128], bf16)
            nc.vector.tensor_copy(ident[:], ident_f[:])
            # multiplicative tril mask for the diagonal chunk of weiT [s,t]:
            # keep (1) where t >= s, 0 where t < s
            tmask_f = consts.tile([128, 128], f32)
            nc.gpsimd.memset(tmask_f[:], 1.0)
            nc.gpsimd.affine_select(
                out=tmask_f[:], in_=tmask_f[:],
                compare_op=mybir.AluOpType.is_ge,
                fill=0.0, base=0, pattern=[[1, 128]], channel_multiplier=-1)
            tmask = consts.tile([128, 128], bf16)
            nc.vector.tensor_copy(tmask[:], tmask_f[:])
            # ones33x64 bf16: K=1 broadcast rows for the 1/l bcast; rows
            # 0/32 used so lhsT base partition matches rc_bf's row
            ones33x64 = consts.tile([33, 64], bf16)
            nc.gpsimd.memset(ones33x64[:], 1.0)
            # sel_pick [33, 2] bf16: col h picks row 32h (used to move the
            # softmax denominators from free-dim-t onto t-partitions)
            sel_pick = consts.tile([33, 2], bf16)
            nc.gpsimd.memset(sel_pick[:], 0.0)
            nc.gpsimd.memset(sel_pick[0:1, 0:1], 1.0)
            nc.gpsimd.memset(sel_pick[32:33, 1:2], 1.0)
            # persistent l tiles (rows 1-31 stay 1.0; sel_pick ignores them)
            l_sbs = [big.tile([33, 512], bf16, name=f'lsb{k}') for k in range(2)]
            for k in range(2):
                nc.gpsimd.memset(l_sbs[k][:], 1.0)

            # ---- weights (DMAs issued inside phase 1 / after it, so the
            # first matmul doesn't sit behind the whole weight load) ----
            wqkv_sb = [consts.tile([128, 3 * DPC], bf16, name=f'wqkv{k}')
                       for k in range(NE)]
            wproj_sb = consts.tile([DPC, E], bf16)

            # ---- persistent activations ----
            qT_sb = [big.tile([128, 512], bf16, name=f'q{j}')
                     for j in range(NTB1)]
            kT_sb = [big.tile([128, 512], bf16, name=f'k{j}')
                     for j in range(NTB1)]
            # v tiles [s, (sc, h, d+1)] per 512-t block: per head 64 dims +
            # ones column (softmax denominator via row 64 of the AV matmul)
            v_sb = [big.tile([128, 4, 2, 65], bf16, name=f'v{j}')
                    for j in range(NTB1)]
            for j in range(NTB1):
                nc.gpsimd.memset(v_sb[j][:, :, :, 64:65], 1.0)

            # ---- phase 1: one 512-t block of QKV projections. q/k accumulate
            # in 'acc' (2-block rotation), v in 'wei'; the v transpose chain
            # is deferred one block (lookahead) so the PE never waits on the
            # vt copy ----
            def emit_qkv_matmuls(j):
                ts = j * 512
                q_ps = ps.tile([128, 512], f32, tag='acc', bufs=4)
                k_ps = ps.tile([128, 512], f32, tag='acc', bufs=4)
                vt_ps = ps.tile([128, 512], f32, tag='wei', bufs=3)
                for k in range(NE):
                    xt_t = work.tile([128, 512], bf16, tag='xt', bufs=8)
                    dma_eng = nc.sync if k % 2 == 0 else nc.scalar
                    if j == 0:
                        # stream this e-tile's weights just ahead of its x
                        dma_eng.dma_start(
                            out=wqkv_sb[k][:],
                            in_=wqkv[k * 128:(k + 1) * 128, :])
                    dma_eng.dma_start(
                        out=xt_t[:], in_=xt[k * 128:(k + 1) * 128, ts:ts + 512])
                    st, sp = (k == 0), (k == NE - 1)
                    nc.tensor.matmul(q_ps[:], wqkv_sb[k][:, 0:128], xt_t[:],
                                     start=st, stop=sp)
                    nc.tensor.matmul(k_ps[:], wqkv_sb[k][:, 128:256], xt_t[:],
                                     start=st, stop=sp)
                    nc.tensor.matmul(vt_ps[:], wqkv_sb[k][:, 256:384], xt_t[:],
                                     start=st, stop=sp)
                nc.vector.tensor_copy(qT_sb[j][:], q_ps[:])
                nc.vector.tensor_copy(kT_sb[j][:], k_ps[:])
                return vt_ps

            def make_v_chain(j, vt_ps):
                vt_sb = work.tile([128, 512], bf16, tag='vt', bufs=2)
                state = {}

                def step_a():
                    nc.vector.tensor_copy(vt_sb[:], vt_ps[:])
                    vtr = ps.tile([128, 512], bf16, tag='wei', bufs=3)
                    for sc in range(4):
                        nc.tensor.transpose(vtr[:, sc * 128:(sc + 1) * 128],
                                            vt_sb[:, sc * 128:(sc + 1) * 128],
                                            ident[:])
                    state['vtr'] = vtr

                def step_b():
                    nc.vector.tensor_copy(
                        v_sb[j][:, :, :, 0:64],
                        state['vtr'].rearrange('p (c h e) -> p c h e',
                                               c=4, h=2))

                return [step_a, step_b]

            # ---- attention for one t-block, with block i-1's epilogue
            # interleaved at (si, h) sub-boundaries ----
            step_queue = []

            def emit_attn_block(b, tb, must_finish=0):
                """must_finish: number of queue-front steps that MUST run
                inside this block (frees av psum banks for block i+2);
                the rest pace at ~1 step per 2 sub-boundaries and may spill
                into later blocks via the global queue."""
                n_si = 4 * (tb + 1)
                av_pss = [ps.tile([65, 512], f32, tag='acc', bufs=4,
                                  name=f'av{b}_{tb}_{h}')
                          for h in range(HPC)]
                pending = None     # (av matmul args) one step behind scores
                popped = 0
                for si in range(n_si):
                    j = b * NTB + si // 4
                    sc = si % 4
                    woff = max(0, (si - 4 * tb) * 128)
                    diag = si >= 4 * tb
                    for h in range(HPC):
                        # pop epilogue steps at sub-boundaries from sub 4 on:
                        # every boundary while the required steps are not yet
                        # done or there is backlog, else every other boundary
                        idx = 2 * si + h - 4
                        if idx >= 0 and step_queue:
                            run = (popped < must_finish
                                   or len(step_queue) > 11
                                   or idx % 2 == 0)
                            if run:
                                step_queue.pop(0)()
                                popped += 1
                        hd = h * 64
                        w_ps = ps.tile([128, 512], f32, tag='wei', bufs=3)
                        nc.tensor.matmul(
                            w_ps[:, woff:512],
                            kT_sb[j][hd:hd + 64, sc * 128:(sc + 1) * 128],
                            qT_sb[b * NTB + tb][hd:hd + 64, woff:512],
                            start=True, stop=True)
                        wt = work.tile([128, 512], bf16, tag='weiT', bufs=8)
                        nc.scalar.activation(wt[:, woff:512], w_ps[:, woff:512],
                                             EXP, scale=SCALE)
                        if diag:
                            nc.vector.tensor_mul(wt[:, woff:woff + 128],
                                                 wt[:, woff:woff + 128],
                                                 tmask[:])
                        if pending is not None:
                            nc.tensor.matmul(**pending)
                        pending = dict(
                            out=av_pss[h][:, woff:512],
                            lhsT=v_sb[j][:, sc, h, :],
                            rhs=wt[:, woff:512],
                            start=(si == 0), stop=(si == n_si - 1),
                            skip_group_check=True)
                if pending is not None:
                    nc.tensor.matmul(**pending)
                # any required steps not yet popped run now (bunched)
                while popped < must_finish and step_queue:
                    step_queue.pop(0)()
                    popped += 1
                return av_pss

            # ---- epilogue steps for one block: pick + backT + bcast + 8 proj
            def make_epi_steps(b, tb, seq, av_pss, final=False):
                t0 = (b * NTB + tb) * 512
                l_sb = l_sbs[seq % 2]
                # only the l copies run eagerly (DVE, right after the last AV);
                # everything else is a step interleaved into the next block
                for h in range(HPC):
                    nc.vector.tensor_copy(l_sb[32 * h:32 * h + 1, :],
                                          av_pss[h][64:65, :])
                avT = work.tile([128, 512], bf16, tag='avT', bufs=2,
                                name=f'avT{seq}')
                rc_t = work.tile([128, 4, 2], bf16, tag='rct', bufs=2)
                rc_bf = work.tile([33, 512], bf16, tag='rcb', bufs=2)

                def step_pick():
                    # denominators -> t-partitions; reciprocal over 8 elems
                    lT_ps = ps.tile([128, 4, 2], f32, tag='wei', bufs=3)
                    for c in range(4):
                        nc.tensor.matmul(lT_ps[:, c, :],
                                         l_sb[:, c * 128:(c + 1) * 128],
                                         sel_pick[:], start=True, stop=True)
                    with nc.allow_low_precision("1/l bf16 within tolerance"):
                        nc.vector.reciprocal(rc_t[:], lT_ps[:])

                def step_backt():
                    rc_ps = ps.tile([33, 512], bf16, tag='wei', bufs=3)
                    for h in range(HPC):
                        for c in range(4):
                            nc.tensor.transpose(
                                rc_ps[32 * h:32 * h + 1,
                                      c * 128:(c + 1) * 128],
                                rc_t[:, c, h:h + 1], ident[:])
                    nc.vector.tensor_copy(rc_bf[:], rc_ps[:])

                def step_bcast():
                    # per-head K=1 broadcast: bc[64h:64h+64, t] = 1/l_h[t];
                    # reads only rc_bf rows 0/32 (rows 1-31 are psum garbage)
                    bc_ps = ps.tile([128, 512], f32, tag='wei', bufs=3)
                    for h in range(HPC):
                        nc.tensor.matmul(bc_ps[64 * h:64 * h + 64, :],
                                         ones33x64[32 * h:32 * h + 1, :],
                                         rc_bf[32 * h:32 * h + 1, :],
                                         start=True, stop=True)
                    bc_sb = work.tile([128, 512], bf16, tag='bcs', bufs=2)
                    nc.vector.tensor_copy(bc_sb[:], bc_ps[:])
                    for h in range(HPC):
                        hd = h * 64
                        nc.vector.tensor_mul(avT[hd:hd + 64, :],
                                             av_pss[h][0:64, :],
                                             bc_sb[hd:hd + 64, :])

                def make_proj(tc4, eb, ytag):
                    def step():
                        y_ps = ps.tile([128, 512], f32, tag=ytag,
                                       bufs=1 if ytag == 'epi' else 3)
                        nc.tensor.matmul(
                            y_ps[:],
                            avT[:, tc4 * 128:(tc4 + 1) * 128],
                            wproj_sb[:, eb * 512:(eb + 1) * 512],
                            start=True, stop=True)
                        y_sb = work.tile([128, 512], bf16, tag='ysb', bufs=6)
                        nc.vector.tensor_copy(y_sb[:], y_ps[:])
                        nc.sync.dma_start(
                            out=y[t0 + tc4 * 128:t0 + (tc4 + 1) * 128,
                                  eb * 512:(eb + 1) * 512],
                            in_=y_sb[:])
                    return step

                step_queue.append(step_pick)
                step_queue.append(step_backt)
                step_queue.append(step_bcast)
                k = 0
                for tc4 in range(4):
                    for eb in range(2):
                        # the final epilogue has no host block; alternate its
                        # y banks with the now-idle wei banks so proj matmuls
                        # don't serialize on y copies
                        ytag = 'wei' if (final and k % 2 == 1) else 'epi'
                        step_queue.append(make_proj(tc4, eb, ytag))
                        k += 1

            # phase 1 with one-block v-chain lookahead; block 7's chain is
            # handed to the first attention block as its epi steps
            prev_chain = None
            for j in range(NTB1):
                vt_ps = emit_qkv_matmuls(j)
                if j == 0:
                    # wproj is first needed by the first epilogue (~80us in)
                    nc.sync.dma_start(out=wproj_sb[:], in_=wproj[:])
                if prev_chain is not None:
                    for f in prev_chain:
                        f()
                prev_chain = make_v_chain(j, vt_ps)

            # batches interleaved so no big block's epilogue lands in a
            # tiny tb=0 host mid-run (only at the cold start)
            order = [(b, tb) for tb in range(NTB) for b in range(B)]
            step_queue.extend(prev_chain)
            for seq, (b, tb) in enumerate(order):
                # everything except the previous epilogue's 8 proj steps must
                # run inside this block (frees that epilogue's av psum banks)
                mf = max(0, len(step_queue) - 8)
                av_pss = emit_attn_block(b, tb, must_finish=mf)
                make_epi_steps(b, tb, seq, av_pss,
                               final=(seq == len(order) - 1))
            while step_queue:
                step_queue.pop(0)()

    import concourse.mybir as mybir2
    _split_multi_waits(nc, mybir2)
    return nc


_CACHE = {}


def kernel(x, Wq, Wk, Wv, Wproj, bproj):
    _install_ntff_hook()
    import ml_dtypes
    from concourse.bass_utils import run_bass_kernel_spmd

    bf = ml_dtypes.bfloat16
    x = np.asarray(x, dtype=np.float32)
    Wq = np.asarray(Wq, dtype=np.float32)
    Wk = np.asarray(Wk, dtype=np.float32)
    Wv = np.asarray(Wv, dtype=np.float32)
    Wproj = np.asarray(Wproj, dtype=np.float32)
    bproj = np.asarray(bproj, dtype=np.float32)

    if 'nc' not in _CACHE:
        _CACHE['nc'] = _build_nc()
    nc = _CACHE['nc']

    xT = np.ascontiguousarray(x.reshape(BT, E).T).astype(bf)
    in_maps = []
    for c in range(N_CORES):
        h0 = HPC * c
        wqkv_c = np.concatenate(
            [Wq[h0], Wq[h0 + 1], Wk[h0], Wk[h0 + 1], Wv[h0], Wv[h0 + 1]],
            axis=1)                                         # [E, 384]
        wproj_c = np.ascontiguousarray(Wproj[DPC * c: DPC * (c + 1)])
        in_maps.append({'xt': xT,
                        'wqkv': np.ascontiguousarray(wqkv_c).astype(bf),
                        'wproj': wproj_c.astype(bf)})

    res = run_bass_kernel_spmd(nc, in_maps, list(range(N_CORES)))
    ysum = np.zeros((BT, E), dtype=np.float64)
    for c in range(N_CORES):
        ysum += res.results[c]['y'].astype(np.float64)
    out = (ysum + bproj.astype(np.float64)).astype(np.float32)
    return out.reshape(B, T, E)


# revision 30
# speedup vs baseline: 1.0040x; 1.0040x over previous
"""Multi-head causal attention (B=2, T=2048, E=1024, H=16, D=64) on 8 TRN2
NeuronCores, tensor-parallel over heads (2 heads per core).

All matmul operands are bf16 (same PE rate as fp32r for N>=256 columns, but
no fp32r 4x penalty on the <256-col causally-trimmed matmuls; halves
DMA/SBUF; x and y travel as bf16). Host-simulated precision of the all-bf16
pipeline: ~5e-3 rel err vs the 2e-2 gate. Measured: 200.3us / 4.03e-3
(baseline fp32r version: 257.7us / 2.8e-4).

Dataflow per core (tensor-parallel over heads; host sums the 8 partial y):
  phase 1 per 512-t block: qT/kT/vT = wqkv^T @ xT (PSUM accum over 8
    e-tiles; q/k in 'acc' banks, vT in a 'wei' bank), then one block later
    PE-transpose vT -> v [s, (sc, h, d)] tiles with a ones column (the
    softmax denominator rides row 64 of the AV matmul output). Weight DMAs
    stream inside block 0; wproj loads after phase 1 emission.
  phase 2 per (b, 512-t block), batches interleaved (b0t0, b1t0, b0t1, ...):
    per 128-s tile x 2 heads: weiT[s,t] = kT^T q (diagonal tiles compute
    only cols >= woff), Exp on ACT (scale=E^-0.5) -> bf16 wt, multiplicative
    tril mask (DVE) on the diagonal chunk, avT[65, t] += [v|1]^T @ wt.
  epilogue per block, decomposed into 11 steps run from a global queue at
    the NEXT blocks' (si, h) sub-boundaries (~1 step per 2 boundaries,
    spilling across blocks; the 3 chain steps must run in the immediate
    next block to free av banks):
      pick: l rows -> t-partitions via a [33,2]-selector matmul, then a DVE
        reciprocal over 8 elements (not 512 -- DVE cost is free-size-bound);
      backT: 8 tiny PE transposes put 1/l back on rows 0/32; bcast: per-head
        K=1 ones matmul broadcasts 1/l over 64 d-partitions (reads only the
        written rows -- rows 1-31 of rc_ps are psum garbage, 0*NaN=NaN);
      avT = av * bc on DVE (bf16), 8 proj matmuls y[t,e] = avT^T @ wproj,
        DVE psum->bf16 copies, DMA out. The final (hostless) epilogue
        alternates its y banks with the idle wei banks.
PSUM banks: av(acc)=4 (2 heads x 2 blocks in flight) + wei=3 + epi(y)=1 = 8.

Infra notes: this container's walrus accepts at most ONE semaphore wait
per instruction (_split_multi_waits hoists extras onto EventSemaphores);
custom-DVE ops / gpsimd partition_broadcast do not compile ("ISA wrong
length"); GPSIMD (Pool) cannot access PSUM and rejects TensorTensor ops
entirely (memset/affine_select only); DMA cannot touch PSUM; non-matmul
engine APs need 32-partition-aligned starts (hence rows 0/32 for l);
matmul lhsT and rhs must share their base partition. The ACT engine must
run a single function (Exp) -- mixing in Copy/Reciprocal thrashes the
activation table at ~1.3us per swap. The PE runs 1.2GHz cold / 2.4GHz
after ~3-4us of sustained work; phase 1 sustains 216ns per 512-col matmul,
the attention phase levels at ~320-335ns (engine-concurrency effect not
worth more gap-chasing below ~300ns granularity).
"""
import sys
import types

import numpy as np

B, T, E, H, D = 2, 2048, 1024, 16, 64
N_CORES = 8
HPC = H // N_CORES          # heads per core = 2
BT = B * T                  # 4096
DPC = HPC * D               # 128 head-dims per core
SCALE = 1.0 / float(np.sqrt(E))  # NOTE: reference scales by E**-0.5

# y copies all on DVE (GPSIMD cannot read PSUM); mask muls go to GpSimd
Y_COPY_ON_GPSIMD = (False,) * 8


def _install_ntff_hook():
    if 'antenv.axon_hooks' in sys.modules:
        return
    try:
        sys.path.insert(0, '/root/.axon_site')
        from trn_agent_boot.trn_boot import _ntff_profile_via_ctypes
        hook = _ntff_profile_via_ctypes('/opt/axon/libaxon_pjrt.so')
        mod = types.ModuleType('antenv.axon_hooks')
        mod.get_axon_ntff_profile_hook = lambda: hook
        mod.set_axon_ntff_profile_hook = lambda h: None
        sys.modules['antenv.axon_hooks'] = mod
    except Exception:
        pass


def _split_multi_waits(nc, mybir):
    """This walrus build rejects >1 sync-wait per instruction. Hoist extra
    waits onto EventSemaphore instructions on the same engine just before."""
    for f in nc.m.functions:
        for bb in f.blocks:
            new_insts = []
            changed = False
            for inst in bb.instructions:
                si = inst.sync_info
                if si is not None and len(si.on_wait) > 1:
                    extra = list(si.on_wait[:-1])
                    keep = si.on_wait[-1]
                    for w in extra:
                        ev = mybir.InstEventSemaphore(
                            name=f"I-{nc.next_id()}", ins=[], outs=[])
                        ev.engine = inst.engine
                        ev.sync_info = mybir.SyncInfo(on_wait=[w], on_update=[])
                        new_insts.append(ev)
                    del si.on_wait[:]
                    si.on_wait.append(keep)
                    changed = True
                new_insts.append(inst)
            if changed:
                bb.instructions = new_insts


def _build_nc():
    import concourse.bass as bass
    import concourse.mybir as mybir
    import concourse.tile as tile
    from concourse.masks import make_identity

    f32 = mybir.dt.float32
    bf16 = mybir.dt.bfloat16
    EXP = mybir.ActivationFunctionType.Exp
    DIV = mybir.AluOpType.divide

    nc = bass.Bass('TRN2', num_devices=N_CORES)
    xt = nc.dram_tensor('xt', [E, BT], bf16, kind='ExternalInput')
    wqkv = nc.dram_tensor('wqkv', [E, 3 * DPC], bf16, kind='ExternalInput')
    wproj = nc.dram_tensor('wproj', [DPC, E], bf16, kind='ExternalInput')
    y = nc.dram_tensor('y', [BT, E], bf16, kind='ExternalOutput')

    NTB1 = BT // 512            # 8 t-blocks in phase 1
    NE = E // 128               # 8 e-tiles
    NTB = T // 512              # 4 t-blocks per batch in phase 2

    with tile.TileContext(nc) as tc:
        with tc.tile_pool(name='consts', bufs=1) as consts, \
             tc.tile_pool(name='big', bufs=1) as big, \
             tc.tile_pool(name='work', bufs=1) as work, \
             tc.tile_pool(name='ps', bufs=1, space='PSUM') as ps:

            # ---- constants ----
            ident_f = consts.tile([128, 128], f32)
            make_identity(nc, ident_f)
            ident = consts.tile([128, 128], bf16)
            nc.vector.tensor_copy(ident[:], ident_f[:])
            # multiplicative tril mask for the diagonal chunk of weiT [s,t]:
            # keep (1) where t >= s, 0 where t < s
            tmask_f = consts.tile([128, 128], f32)
            nc.gpsimd.memset(tmask_f[:], 1.0)
            nc.gpsimd.affine_select(
                out=tmask_f[:], in_=tmask_f[:],
                compare_op=mybir.AluOpType.is_ge,
                fill=0.0, base=0, pattern=[[1, 128]], channel_multiplier=-1)
            tmask = consts.tile([128, 128], bf16)
            nc.vector.tensor_copy(tmask[:], tmask_f[:])
            # ones33x64 bf16: K=1 broadcast rows for the 1/l bcast; rows
            # 0/32 used so lhsT base partition matches rc_bf's row
            ones33x64 = consts.tile([33, 64], bf16)
            nc.gpsimd.memset(ones33x64[:], 1.0)
            # sel_pick [33, 2] bf16: col h picks row 32h (used to move the
            # softmax denominators from free-dim-t onto t-partitions)
            sel_pick = consts.tile([33, 2], bf16)
            nc.gpsimd.memset(sel_pick[:], 0.0)
            nc.gpsimd.memset(sel_pick[0:1, 0:1], 1.0)
            nc.gpsimd.memset(sel_pick[32:33, 1:2], 1.0)
            # persistent l tiles (rows 1-31 stay 1.0; sel_pick ignores them)
            l_sbs = [big.tile([33, 512], bf16, name=f'lsb{k}') for k in range(2)]
            for k in range(2):
                nc.gpsimd.memset(l_sbs[k][:], 1.0)

            # ---- weights (DMAs issued inside phase 1 / after it, so the
            # first matmul doesn't sit behind the whole weight load) ----
            wqkv_sb = [consts.tile([128, 3 * DPC], bf16, name=f'wqkv{k}')
                       for k in range(NE)]
            wproj_sb = consts.tile([DPC, E], bf16)

            # ---- persistent activations ----
            qT_sb = [big.tile([128, 512], bf16, name=f'q{j}')
                     for j in range(NTB1)]
            kT_sb = [big.tile([128, 512], bf16, name=f'k{j}')
                     for j in range(NTB1)]
            # v tiles [s, (sc, h, d+1)] per 512-t block: per head 64 dims +
            # ones column (softmax denominator via row 64 of the AV matmul)
            v_sb = [big.tile([128, 4, 2, 65], bf16, name=f'v{j}')
                    for j in range(NTB1)]
            for j in range(NTB1):
                nc.gpsimd.memset(v_sb[j][:, :, :, 64:65], 1.0)

            # ---- phase 1: one 512-t block of QKV projections. q/k accumulate
            # in 'acc' (2-block rotation), v in 'wei'; the v transpose chain
            # is deferred one block (lookahead) so the PE never waits on the
            # vt copy ----
            def emit_qkv_matmuls(j):
                ts = j * 512
                q_ps = ps.tile([128, 512], f32, tag='acc', bufs=4)
                k_ps = ps.tile([128, 512], f32, tag='acc', bufs=4)
                vt_ps = ps.tile([128, 512], f32, tag='wei', bufs=3)
                for k in range(NE):
                    xt_t = work.tile([128, 512], bf16, tag='xt', bufs=12)
                    dma_eng = nc.sync if k % 2 == 0 else nc.scalar
                    if j == 0:
                        # stream this e-tile's weights just ahead of its x
                        dma_eng.dma_start(
                            out=wqkv_sb[k][:],
                            in_=wqkv[k * 128:(k + 1) * 128, :])
                    dma_eng.dma_start(
                        out=xt_t[:], in_=xt[k * 128:(k + 1) * 128, ts:ts + 512])
                    st, sp = (k == 0), (k == NE - 1)
                    nc.tensor.matmul(q_ps[:], wqkv_sb[k][:, 0:128], xt_t[:],
                                     start=st, stop=sp)
                    nc.tensor.matmul(k_ps[:], wqkv_sb[k][:, 128:256], xt_t[:],
                                     start=st, stop=sp)
                    nc.tensor.matmul(vt_ps[:], wqkv_sb[k][:, 256:384], xt_t[:],
                                     start=st, stop=sp)
                nc.vector.tensor_copy(qT_sb[j][:], q_ps[:])
                nc.vector.tensor_copy(kT_sb[j][:], k_ps[:])
                return vt_ps

            def make_v_chain(j, vt_ps):
                vt_sb = work.tile([128, 512], bf16, tag='vt', bufs=2)
                state = {}

                def step_a():
                    nc.vector.tensor_copy(vt_sb[:], vt_ps[:])
                    vtr = ps.tile([128, 512], bf16, tag='wei', bufs=3)
                    for sc in range(4):
                        nc.tensor.transpose(vtr[:, sc * 128:(sc + 1) * 128],
                                            vt_sb[:, sc * 128:(sc + 1) * 128],
                                            ident[:])
                    state['vtr'] = vtr

                def step_b():
                    nc.vector.tensor_copy(
                        v_sb[j][:, :, :, 0:64],
                        state['vtr'].rearrange('p (c h e) -> p c h e',
                                               c=4, h=2))

                return [step_a, step_b]

            # ---- attention for one t-block, with block i-1's epilogue
            # interleaved at (si, h) sub-boundaries ----
            step_queue = []

            def emit_attn_block(b, tb, must_finish=0):
                """must_finish: number of queue-front steps that MUST run
                inside this block (frees av psum banks for block i+2);
                the rest pace at ~1 step per 2 sub-boundaries and may spill
                into later blocks via the global queue."""
                n_si = 4 * (tb + 1)
                av_pss = [ps.tile([65, 512], f32, tag='acc', bufs=4,
                                  name=f'av{b}_{tb}_{h}')
                          for h in range(HPC)]
                pending = None     # (av matmul args) one step behind scores
                popped = 0
                for si in range(n_si):
                    j = b * NTB + si // 4
                    sc = si % 4
                    woff = max(0, (si - 4 * tb) * 128)
                    diag = si >= 4 * tb
                    for h in range(HPC):
                        # pop epilogue steps at sub-boundaries from sub 4 on:
                        # every boundary while the required steps are not yet
                        # done or there is backlog, else every other boundary
                        idx = 2 * si + h - 4
                        if idx >= 0 and step_queue:
                            run = (popped < must_finish
                                   or len(step_queue) > 8
                                   or idx % 2 == 0)
                            if run:
                                step_queue.pop(0)()
                                popped += 1
                        hd = h * 64
                        w_ps = ps.tile([128, 512], f32, tag='wei', bufs=3)
                        nc.tensor.matmul(
                            w_ps[:, woff:512],
                            kT_sb[j][hd:hd + 64, sc * 128:(sc + 1) * 128],
                            qT_sb[b * NTB + tb][hd:hd + 64, woff:512],
                            start=True, stop=True)
                        wt = work.tile([128, 512], bf16, tag='weiT', bufs=12)
                        nc.scalar.activation(wt[:, woff:512], w_ps[:, woff:512],
                                             EXP, scale=SCALE)
                        if diag:
                            nc.vector.tensor_mul(wt[:, woff:woff + 128],
                                                 wt[:, woff:woff + 128],
                                                 tmask[:])
                        if pending is not None:
                            nc.tensor.matmul(**pending)
                        pending = dict(
                            out=av_pss[h][:, woff:512],
                            lhsT=v_sb[j][:, sc, h, :],
                            rhs=wt[:, woff:512],
                            start=(si == 0), stop=(si == n_si - 1),
                            skip_group_check=True)
                if pending is not None:
                    nc.tensor.matmul(**pending)
                # any required steps not yet popped run now (bunched)
                while popped < must_finish and step_queue:
                    step_queue.pop(0)()
                    popped += 1
                return av_pss

            # ---- epilogue steps for one block: pick + backT + bcast + 8 proj
            def make_epi_steps(b, tb, seq, av_pss, final=False):
                t0 = (b * NTB + tb) * 512
                l_sb = l_sbs[seq % 2]
                # only the l copies run eagerly (DVE, right after the last AV);
                # everything else is a step interleaved into the next block
                for h in range(HPC):
                    nc.vector.tensor_copy(l_sb[32 * h:32 * h + 1, :],
                                          av_pss[h][64:65, :])
                avT = work.tile([128, 512], bf16, tag='avT', bufs=2,
                                name=f'avT{seq}')
                rc_t = work.tile([128, 4, 2], bf16, tag='rct', bufs=2)
                rc_bf = work.tile([33, 512], bf16, tag='rcb', bufs=2)

                def step_pick():
                    # denominators -> t-partitions; reciprocal over 8 elems
                    lT_ps = ps.tile([128, 4, 2], f32, tag='wei', bufs=3)
                    for c in range(4):
                        nc.tensor.matmul(lT_ps[:, c, :],
                                         l_sb[:, c * 128:(c + 1) * 128],
                                         sel_pick[:], start=True, stop=True)
                    with nc.allow_low_precision("1/l bf16 within tolerance"):
                        nc.vector.reciprocal(rc_t[:], lT_ps[:])

                def step_backt():
                    rc_ps = ps.tile([33, 512], bf16, tag='wei', bufs=3)
                    for h in range(HPC):
                        for c in range(4):
                            nc.tensor.transpose(
                                rc_ps[32 * h:32 * h + 1,
                                      c * 128:(c + 1) * 128],
                                rc_t[:, c, h:h + 1], ident[:])
                    nc.vector.tensor_copy(rc_bf[:], rc_ps[:])

                def step_bcast():
                    # per-head K=1 broadcast: bc[64h:64h+64, t] = 1/l_h[t];
                    # reads only rc_bf rows 0/32 (rows 1-31 are psum garbage)
                    bc_ps = ps.tile([128, 512], f32, tag='wei', bufs=3)
                    for h in range(HPC):
                        nc.tensor.matmul(bc_ps[64 * h:64 * h + 64, :],
                                         ones33x64[32 * h:32 * h + 1, :],
                                         rc_bf[32 * h:32 * h + 1, :],
                                         start=True, stop=True)
                    bc_sb = work.tile([128, 512], bf16, tag='bcs', bufs=2)
                    nc.vector.tensor_copy(bc_sb[:], bc_ps[:])
                    for h in range(HPC):
                        hd = h * 64
                        nc.vector.tensor_mul(avT[hd:hd + 64, :],
                                             av_pss[h][0:64, :],
                                             bc_sb[hd:hd + 64, :])

                def make_proj(tc4, eb, ytag):
                    def step():
                        y_ps = ps.tile([128, 512], f32, tag=ytag,
                                       bufs=1 if ytag == 'epi' else 3)
                        nc.tensor.matmul(
                            y_ps[:],
                            avT[:, tc4 * 128:(tc4 + 1) * 128],
                            wproj_sb[:, eb * 512:(eb + 1) * 512],
                            start=True, stop=True)
                        y_sb = work.tile([128, 512], bf16, tag='ysb', bufs=8)
                        nc.vector.tensor_copy(y_sb[:], y_ps[:])
                        nc.sync.dma_start(
                            out=y[t0 + tc4 * 128:t0 + (tc4 + 1) * 128,
                                  eb * 512:(eb + 1) * 512],
                            in_=y_sb[:])
                    return step

                step_queue.append(step_pick)
                step_queue.append(step_backt)
                step_queue.append(step_bcast)
                k = 0
                for tc4 in range(4):
                    for eb in range(2):
                        # the final epilogue has no host block; alternate its
                        # y banks with the now-idle wei banks so proj matmuls
                        # don't serialize on y copies
                        ytag = 'wei' if (final and k % 2 == 1) else 'epi'
                        step_queue.append(make_proj(tc4, eb, ytag))
                        k += 1

            # phase 1 with one-block v-chain lookahead; block 7's chain is
            # handed to the first attention block as its epi steps
            prev_chain = None
            for j in range(NTB1):
                vt_ps = emit_qkv_matmuls(j)
                if j == 0:
                    # wproj is first needed by the first epilogue (~80us in)
                    nc.sync.dma_start(out=wproj_sb[:], in_=wproj[:])
                if prev_chain is not None:
                    for f in prev_chain:
                        f()
                prev_chain = make_v_chain(j, vt_ps)

            # batches interleaved so no big block's epilogue lands in a
            # tiny tb=0 host mid-run (only at the cold start)
            order = [(b, tb) for tb in range(NTB) for b in range(B)]
            step_queue.extend(prev_chain)
            for seq, (b, tb) in enumerate(order):
                # everything except the previous epilogue's 8 proj steps must
                # run inside this block (frees that epilogue's av psum banks)
                mf = max(0, len(step_queue) - 8)
                av_pss = emit_attn_block(b, tb, must_finish=mf)
                make_epi_steps(b, tb, seq, av_pss,
                               final=(seq == len(order) - 1))
            while step_queue:
                step_queue.pop(0)()

    import concourse.mybir as mybir2
    _split_multi_waits(nc, mybir2)
    return nc


_CACHE = {}


def kernel(x, Wq, Wk, Wv, Wproj, bproj):
    _install_ntff_hook()
    import ml_dtypes
    from concourse.bass_utils import run_bass_kernel_spmd

    bf = ml_dtypes.bfloat16
    x = np.asarray(x, dtype=np.float32)
    Wq = np.asarray(Wq, dtype=np.float32)
    Wk = np.asarray(Wk, dtype=np.float32)
    Wv = np.asarray(Wv, dtype=np.float32)
    Wproj = np.asarray(Wproj, dtype=np.float32)
    bproj = np.asarray(bproj, dtype=np.float32)

    if 'nc' not in _CACHE:
        _CACHE['nc'] = _build_nc()
    nc = _CACHE['nc']

    xT = np.ascontiguousarray(x.reshape(BT, E).T).astype(bf)
    in_maps = []
    for c in range(N_CORES):
        h0 = HPC * c
        wqkv_c = np.concatenate(
            [Wq[h0], Wq[h0 + 1], Wk[h0], Wk[h0 + 1], Wv[h0], Wv[h0 + 1]],
            axis=1)                                         # [E, 384]
        wproj_c = np.ascontiguousarray(Wproj[DPC * c: DPC * (c + 1)])
        in_maps.append({'xt': xT,
                        'wqkv': np.ascontiguousarray(wqkv_c).astype(bf),
                        'wproj': wproj_c.astype(bf)})

    res = run_bass_kernel_spmd(nc, in_maps, list(range(N_CORES)))
    ysum = np.zeros((BT, E), dtype=np.float64)
    for c in range(N_CORES):
        ysum += res.results[c]['y'].astype(np.float64)
    out = (ysum + bproj.astype(np.float64)).astype(np.float32)
    return out.reshape(B, T, E)


# revision 31
# speedup vs baseline: 1.0153x; 1.0112x over previous
"""Multi-head causal attention (B=2, T=2048, E=1024, H=16, D=64) on 8 TRN2
NeuronCores, tensor-parallel over heads (2 heads per core).

All matmul operands are bf16 (same PE rate as fp32r for N>=256 columns, but
no fp32r 4x penalty on the <256-col causally-trimmed matmuls; halves
DMA/SBUF; x and y travel as bf16). Host-simulated precision of the all-bf16
pipeline: ~5e-3 rel err vs the 2e-2 gate. Measured: 200.3us / 4.03e-3
(baseline fp32r version: 257.7us / 2.8e-4).

Dataflow per core (tensor-parallel over heads; host sums the 8 partial y):
  phase 1 per 512-t block: qT/kT/vT = wqkv^T @ xT (PSUM accum over 8
    e-tiles; q/k in 'acc' banks, vT in a 'wei' bank), then one block later
    PE-transpose vT -> v [s, (sc, h, d)] tiles with a ones column (the
    softmax denominator rides row 64 of the AV matmul output). Weight DMAs
    stream inside block 0; wproj loads after phase 1 emission.
  phase 2 per (b, 512-t block), batches interleaved (b0t0, b1t0, b0t1, ...):
    per 128-s tile x 2 heads: weiT[s,t] = kT^T q (diagonal tiles compute
    only cols >= woff), Exp on ACT (scale=E^-0.5) -> bf16 wt, multiplicative
    tril mask (DVE) on the diagonal chunk, avT[65, t] += [v|1]^T @ wt.
  epilogue per block, decomposed into 11 steps run from a global queue at
    the NEXT blocks' (si, h) sub-boundaries (~1 step per 2 boundaries,
    spilling across blocks; the 3 chain steps must run in the immediate
    next block to free av banks):
      pick: l rows -> t-partitions via a [33,2]-selector matmul, then a DVE
        reciprocal over 8 elements (not 512 -- DVE cost is free-size-bound);
      backT: 8 tiny PE transposes put 1/l back on rows 0/32; bcast: per-head
        K=1 ones matmul broadcasts 1/l over 64 d-partitions (reads only the
        written rows -- rows 1-31 of rc_ps are psum garbage, 0*NaN=NaN);
      avT = av * bc on DVE (bf16), 8 proj matmuls y[t,e] = avT^T @ wproj,
        DVE psum->bf16 copies, DMA out. The final (hostless) epilogue
        alternates its y banks with the idle wei banks.
PSUM banks: av(acc)=4 (2 heads x 2 blocks in flight) + wei=3 + epi(y)=1 = 8.

Infra notes: this container's walrus accepts at most ONE semaphore wait
per instruction (_split_multi_waits hoists extras onto EventSemaphores);
custom-DVE ops / gpsimd partition_broadcast do not compile ("ISA wrong
length"); GPSIMD (Pool) cannot access PSUM and rejects TensorTensor ops
entirely (memset/affine_select only); DMA cannot touch PSUM; non-matmul
engine APs need 32-partition-aligned starts (hence rows 0/32 for l);
matmul lhsT and rhs must share their base partition. The ACT engine must
run a single function (Exp) -- mixing in Copy/Reciprocal thrashes the
activation table at ~1.3us per swap. The PE runs 1.2GHz cold / 2.4GHz
after ~3-4us of sustained work; phase 1 sustains 216ns per 512-col matmul,
the attention phase levels at ~320-335ns (engine-concurrency effect not
worth more gap-chasing below ~300ns granularity).
"""
import sys
import types

import numpy as np

B, T, E, H, D = 2, 2048, 1024, 16, 64
N_CORES = 8
HPC = H // N_CORES          # heads per core = 2
BT = B * T                  # 4096
DPC = HPC * D               # 128 head-dims per core
SCALE = 1.0 / float(np.sqrt(E))  # NOTE: reference scales by E**-0.5

# y copies all on DVE (GPSIMD cannot read PSUM); mask muls go to GpSimd
Y_COPY_ON_GPSIMD = (False,) * 8


def _install_ntff_hook():
    if 'antenv.axon_hooks' in sys.modules:
        return
    try:
        sys.path.insert(0, '/root/.axon_site')
        from trn_agent_boot.trn_boot import _ntff_profile_via_ctypes
        hook = _ntff_profile_via_ctypes('/opt/axon/libaxon_pjrt.so')
        mod = types.ModuleType('antenv.axon_hooks')
        mod.get_axon_ntff_profile_hook = lambda: hook
        mod.set_axon_ntff_profile_hook = lambda h: None
        sys.modules['antenv.axon_hooks'] = mod
    except Exception:
        pass


def _split_multi_waits(nc, mybir):
    """This walrus build rejects >1 sync-wait per instruction. Hoist extra
    waits onto EventSemaphore instructions on the same engine just before."""
    for f in nc.m.functions:
        for bb in f.blocks:
            new_insts = []
            changed = False
            for inst in bb.instructions:
                si = inst.sync_info
                if si is not None and len(si.on_wait) > 1:
                    extra = list(si.on_wait[:-1])
                    keep = si.on_wait[-1]
                    for w in extra:
                        ev = mybir.InstEventSemaphore(
                            name=f"I-{nc.next_id()}", ins=[], outs=[])
                        ev.engine = inst.engine
                        ev.sync_info = mybir.SyncInfo(on_wait=[w], on_update=[])
                        new_insts.append(ev)
                    del si.on_wait[:]
                    si.on_wait.append(keep)
                    changed = True
                new_insts.append(inst)
            if changed:
                bb.instructions = new_insts


def _build_nc():
    import concourse.bass as bass
    import concourse.mybir as mybir
    import concourse.tile as tile
    from concourse.masks import make_identity

    f32 = mybir.dt.float32
    bf16 = mybir.dt.bfloat16
    EXP = mybir.ActivationFunctionType.Exp
    DIV = mybir.AluOpType.divide

    nc = bass.Bass('TRN2', num_devices=N_CORES)
    xt = nc.dram_tensor('xt', [E, BT], bf16, kind='ExternalInput')
    wqkv = nc.dram_tensor('wqkv', [E, 3 * DPC], bf16, kind='ExternalInput')
    wproj = nc.dram_tensor('wproj', [DPC, E], bf16, kind='ExternalInput')
    y = nc.dram_tensor('y', [BT, E], bf16, kind='ExternalOutput')

    NTB1 = BT // 512            # 8 t-blocks in phase 1
    NE = E // 128               # 8 e-tiles
    NTB = T // 512              # 4 t-blocks per batch in phase 2

    with tile.TileContext(nc) as tc:
        with tc.tile_pool(name='consts', bufs=1) as consts, \
             tc.tile_pool(name='big', bufs=1) as big, \
             tc.tile_pool(name='work', bufs=1) as work, \
             tc.tile_pool(name='ps', bufs=1, space='PSUM') as ps:

            # ---- constants ----
            ident_f = consts.tile([128, 128], f32)
            make_identity(nc, ident_f)
            ident = consts.tile([128, 128], bf16)
            nc.vector.tensor_copy(ident[:], ident_f[:])
            # multiplicative tril mask for the diagonal chunk of weiT [s,t]:
            # keep (1) where t >= s, 0 where t < s
            tmask_f = consts.tile([128, 128], f32)
            nc.gpsimd.memset(tmask_f[:], 1.0)
            nc.gpsimd.affine_select(
                out=tmask_f[:], in_=tmask_f[:],
                compare_op=mybir.AluOpType.is_ge,
                fill=0.0, base=0, pattern=[[1, 128]], channel_multiplier=-1)
            tmask = consts.tile([128, 128], bf16)
            nc.vector.tensor_copy(tmask[:], tmask_f[:])
            # ones33x64 bf16: K=1 broadcast rows for the 1/l bcast; rows
            # 0/32 used so lhsT base partition matches rc_bf's row
            ones33x64 = consts.tile([33, 64], bf16)
            nc.gpsimd.memset(ones33x64[:], 1.0)
            # sel_pick [33, 2] bf16: col h picks row 32h (used to move the
            # softmax denominators from free-dim-t onto t-partitions)
            sel_pick = consts.tile([33, 2], bf16)
            nc.gpsimd.memset(sel_pick[:], 0.0)
            nc.gpsimd.memset(sel_pick[0:1, 0:1], 1.0)
            nc.gpsimd.memset(sel_pick[32:33, 1:2], 1.0)
            # persistent l tiles (rows 1-31 stay 1.0; sel_pick ignores them)
            l_sbs = [big.tile([33, 512], bf16, name=f'lsb{k}') for k in range(2)]
            for k in range(2):
                nc.gpsimd.memset(l_sbs[k][:], 1.0)

            # ---- weights (DMAs issued inside phase 1 / after it, so the
            # first matmul doesn't sit behind the whole weight load) ----
            wqkv_sb = [consts.tile([128, 3 * DPC], bf16, name=f'wqkv{k}')
                       for k in range(NE)]
            wproj_sb = consts.tile([DPC, E], bf16)

            # ---- persistent activations ----
            qT_sb = [big.tile([128, 512], bf16, name=f'q{j}')
                     for j in range(NTB1)]
            kT_sb = [big.tile([128, 512], bf16, name=f'k{j}')
                     for j in range(NTB1)]
            # v tiles [s, (sc, h, d+1)] per 512-t block: per head 64 dims +
            # ones column (softmax denominator via row 64 of the AV matmul)
            v_sb = [big.tile([128, 4, 2, 65], bf16, name=f'v{j}')
                    for j in range(NTB1)]
            for j in range(NTB1):
                nc.gpsimd.memset(v_sb[j][:, :, :, 64:65], 1.0)

            # ---- phase 1: one 512-t block of QKV projections. q/k accumulate
            # in 'acc' (2-block rotation), v in 'wei'; the v transpose chain
            # is deferred one block (lookahead) so the PE never waits on the
            # vt copy ----
            def emit_qkv_matmuls(j):
                ts = j * 512
                q_ps = ps.tile([128, 512], f32, tag='acc', bufs=4)
                k_ps = ps.tile([128, 512], f32, tag='acc', bufs=4)
                vt_ps = ps.tile([128, 512], f32, tag='wei', bufs=3)
                for k in range(NE):
                    xt_t = work.tile([128, 512], bf16, tag='xt', bufs=12)
                    dma_eng = nc.sync if k % 2 == 0 else nc.scalar
                    if j == 0:
                        # stream this e-tile's weights just ahead of its x
                        dma_eng.dma_start(
                            out=wqkv_sb[k][:],
                            in_=wqkv[k * 128:(k + 1) * 128, :])
                    dma_eng.dma_start(
                        out=xt_t[:], in_=xt[k * 128:(k + 1) * 128, ts:ts + 512])
                    st, sp = (k == 0), (k == NE - 1)
                    nc.tensor.matmul(q_ps[:], wqkv_sb[k][:, 0:128], xt_t[:],
                                     start=st, stop=sp)
                    nc.tensor.matmul(k_ps[:], wqkv_sb[k][:, 128:256], xt_t[:],
                                     start=st, stop=sp)
                    nc.tensor.matmul(vt_ps[:], wqkv_sb[k][:, 256:384], xt_t[:],
                                     start=st, stop=sp)
                nc.vector.tensor_copy(qT_sb[j][:], q_ps[:])
                nc.vector.tensor_copy(kT_sb[j][:], k_ps[:])
                return vt_ps

            def make_v_chain(j, vt_ps):
                vt_sb = work.tile([128, 512], bf16, tag='vt', bufs=2)
                state = {}

                def step_a():
                    nc.vector.tensor_copy(vt_sb[:], vt_ps[:])
                    vtr = ps.tile([128, 512], bf16, tag='wei', bufs=3)
                    for sc in range(4):
                        nc.tensor.transpose(vtr[:, sc * 128:(sc + 1) * 128],
                                            vt_sb[:, sc * 128:(sc + 1) * 128],
                                            ident[:])
                    state['vtr'] = vtr

                def step_b():
                    nc.vector.tensor_copy(
                        v_sb[j][:, :, :, 0:64],
                        state['vtr'].rearrange('p (c h e) -> p c h e',
                                               c=4, h=2))

                return [step_a, step_b]

            # ---- attention for one t-block, with block i-1's epilogue
            # interleaved at (si, h) sub-boundaries ----
            step_queue = []

            def emit_attn_block(b, tb, must_finish=0):
                """must_finish: number of queue-front steps that MUST run
                inside this block (frees av psum banks for block i+2);
                the rest pace at ~1 step per 2 sub-boundaries and may spill
                into later blocks via the global queue."""
                n_si = 4 * (tb + 1)
                av_pss = [ps.tile([65, 512], f32, tag='acc', bufs=4,
                                  name=f'av{b}_{tb}_{h}')
                          for h in range(HPC)]
                pending = None     # (av matmul args) one step behind scores
                popped = 0
                for si in range(n_si):
                    j = b * NTB + si // 4
                    sc = si % 4
                    woff = max(0, (si - 4 * tb) * 128)
                    diag = si >= 4 * tb
                    for h in range(HPC):
                        # pop epilogue steps at sub-boundaries from sub 4 on:
                        # every boundary while the required steps are not yet
                        # done or there is backlog, else every other boundary
                        idx = 2 * si + h - 4
                        if idx >= 0 and step_queue:
                            run = (popped < must_finish
                                   or len(step_queue) > 8
                                   or idx % 2 == 0)
                            if run:
                                step_queue.pop(0)()
                                popped += 1
                        hd = h * 64
                        w_ps = ps.tile([128, 512], f32, tag='wei', bufs=3)
                        nc.tensor.matmul(
                            w_ps[:, woff:512],
                            kT_sb[j][hd:hd + 64, sc * 128:(sc + 1) * 128],
                            qT_sb[b * NTB + tb][hd:hd + 64, woff:512],
                            start=True, stop=True)
                        wt = work.tile([128, 512], bf16, tag='weiT', bufs=12)
                        nc.scalar.activation(wt[:, woff:512], w_ps[:, woff:512],
                                             EXP, scale=SCALE)
                        if diag:
                            nc.vector.tensor_mul(wt[:, woff:woff + 128],
                                                 wt[:, woff:woff + 128],
                                                 tmask[:])
                        if pending is not None:
                            nc.tensor.matmul(**pending)
                        pending = dict(
                            out=av_pss[h][:, woff:512],
                            lhsT=v_sb[j][:, sc, h, :],
                            rhs=wt[:, woff:512],
                            start=(si == 0), stop=(si == n_si - 1),
                            skip_group_check=True)
                if pending is not None:
                    nc.tensor.matmul(**pending)
                # any required steps not yet popped run now (bunched)
                while popped < must_finish and step_queue:
                    step_queue.pop(0)()
                    popped += 1
                return av_pss

            # ---- epilogue steps for one block: pick + backT + bcast + 8 proj
            def make_epi_steps(b, tb, seq, av_pss, final=False):
                t0 = (b * NTB + tb) * 512
                l_sb = l_sbs[seq % 2]
                # only the l copies run eagerly (DVE, right after the last AV);
                # everything else is a step interleaved into the next block
                for h in range(HPC):
                    nc.vector.tensor_copy(l_sb[32 * h:32 * h + 1, :],
                                          av_pss[h][64:65, :])
                avT = work.tile([128, 512], bf16, tag='avT', bufs=2,
                                name=f'avT{seq}')
                rc_t = work.tile([128, 4, 2], bf16, tag='rct', bufs=2)
                rc_bf = work.tile([33, 512], bf16, tag='rcb', bufs=2)

                def step_pick():
                    # denominators -> t-partitions; reciprocal over 8 elems
                    lT_ps = ps.tile([128, 4, 2], f32, tag='wei', bufs=3)
                    for c in range(4):
                        nc.tensor.matmul(lT_ps[:, c, :],
                                         l_sb[:, c * 128:(c + 1) * 128],
                                         sel_pick[:], start=True, stop=True)
                    with nc.allow_low_precision("1/l bf16 within tolerance"):
                        nc.vector.reciprocal(rc_t[:], lT_ps[:])

                def step_backt():
                    rc_ps = ps.tile([33, 512], bf16, tag='wei', bufs=3)
                    for h in range(HPC):
                        for c in range(4):
                            nc.tensor.transpose(
                                rc_ps[32 * h:32 * h + 1,
                                      c * 128:(c + 1) * 128],
                                rc_t[:, c, h:h + 1], ident[:])
                    nc.vector.tensor_copy(rc_bf[:], rc_ps[:])

                def step_bcast():
                    # per-head K=1 broadcast: bc[64h:64h+64, t] = 1/l_h[t];
                    # reads only rc_bf rows 0/32 (rows 1-31 are psum garbage)
                    bc_ps = ps.tile([128, 512], f32, tag='wei', bufs=3)
                    for h in range(HPC):
                        nc.tensor.matmul(bc_ps[64 * h:64 * h + 64, :],
                                         ones33x64[32 * h:32 * h + 1, :],
                                         rc_bf[32 * h:32 * h + 1, :],
                                         start=True, stop=True)
                    bc_sb = work.tile([128, 512], bf16, tag='bcs', bufs=2)
                    nc.vector.tensor_copy(bc_sb[:], bc_ps[:])
                    for h in range(HPC):
                        hd = h * 64
                        nc.vector.tensor_mul(avT[hd:hd + 64, :],
                                             av_pss[h][0:64, :],
                                             bc_sb[hd:hd + 64, :])

                def make_proj(tc4, eb, ytag):
                    def step():
                        y_ps = ps.tile([128, 512], f32, tag=ytag,
                                       bufs=1 if ytag == 'epi' else 3)
                        nc.tensor.matmul(
                            y_ps[:],
                            avT[:, tc4 * 128:(tc4 + 1) * 128],
                            wproj_sb[:, eb * 512:(eb + 1) * 512],
                            start=True, stop=True)
                        y_sb = work.tile([128, 512], bf16, tag='ysb', bufs=8)
                        nc.vector.tensor_copy(y_sb[:], y_ps[:])
                        nc.sync.dma_start(
                            out=y[t0 + tc4 * 128:t0 + (tc4 + 1) * 128,
                                  eb * 512:(eb + 1) * 512],
                            in_=y_sb[:])
                    return step

                step_queue.append(step_pick)
                step_queue.append(step_backt)
                step_queue.append(step_bcast)
                k = 0
                for tc4 in range(4):
                    for eb in range(2):
                        # the final epilogue has no host block; alternate its
                        # y banks with the now-idle wei banks so proj matmuls
                        # don't serialize on y copies
                        ytag = 'wei' if (final and k % 2 == 1) else 'epi'
                        step_queue.append(make_proj(tc4, eb, ytag))
                        k += 1

            # phase 1 with one-block v-chain lookahead; block 7's chain is
            # handed to the first attention block as its epi steps
            prev_chain = None
            for j in range(NTB1):
                vt_ps = emit_qkv_matmuls(j)
                if j == 0:
                    # wproj is first needed by the first epilogue (~80us in)
                    nc.sync.dma_start(out=wproj_sb[:], in_=wproj[:])
                if prev_chain is not None:
                    for f in prev_chain:
                        f()
                prev_chain = make_v_chain(j, vt_ps)

            # batches interleaved so no big block's epilogue lands in a
            # tiny tb=0 host mid-run (only at the cold start)
            order = [(b, tb) for tb in range(NTB) for b in range(B)]
            step_queue.extend(prev_chain)
            for seq, (b, tb) in enumerate(order):
                # everything except the previous epilogue's 8 proj steps must
                # run inside this block (frees that epilogue's av psum banks);
                # the last block drains the whole queue so the hostless final
                # flush holds only its own epilogue
                mf = max(0, len(step_queue) - 8)
                if seq == len(order) - 1:
                    mf = len(step_queue)
                av_pss = emit_attn_block(b, tb, must_finish=mf)
                make_epi_steps(b, tb, seq, av_pss,
                               final=(seq == len(order) - 1))
            while step_queue:
                step_queue.pop(0)()

    import concourse.mybir as mybir2
    _split_multi_waits(nc, mybir2)
    return nc


_CACHE = {}


def kernel(x, Wq, Wk, Wv, Wproj, bproj):
    _install_ntff_hook()
    import ml_dtypes
    from concourse.bass_utils import run_bass_kernel_spmd

    bf = ml_dtypes.bfloat16
    x = np.asarray(x, dtype=np.float32)
    Wq = np.asarray(Wq, dtype=np.float32)
    Wk = np.asarray(Wk, dtype=np.float32)
    Wv = np.asarray(Wv, dtype=np.float32)
    Wproj = np.asarray(Wproj, dtype=np.float32)
    bproj = np.asarray(bproj, dtype=np.float32)

    if 'nc' not in _CACHE:
        _CACHE['nc'] = _build_nc()
    nc = _CACHE['nc']

    xT = np.ascontiguousarray(x.reshape(BT, E).T).astype(bf)
    in_maps = []
    for c in range(N_CORES):
        h0 = HPC * c
        wqkv_c = np.concatenate(
            [Wq[h0], Wq[h0 + 1], Wk[h0], Wk[h0 + 1], Wv[h0], Wv[h0 + 1]],
            axis=1)                                         # [E, 384]
        wproj_c = np.ascontiguousarray(Wproj[DPC * c: DPC * (c + 1)])
        in_maps.append({'xt': xT,
                        'wqkv': np.ascontiguousarray(wqkv_c).astype(bf),
                        'wproj': wproj_c.astype(bf)})

    res = run_bass_kernel_spmd(nc, in_maps, list(range(N_CORES)))
    ysum = np.zeros((BT, E), dtype=np.float64)
    for c in range(N_CORES):
        ysum += res.results[c]['y'].astype(np.float64)
    out = (ysum + bproj.astype(np.float64)).astype(np.float32)
    return out.reshape(B, T, E)
